# revision 5
# baseline (speedup 1.0000x reference)
"""Trainium2 Bass kernel for ChannelAttentionModel (segment avg/max -> tiny MLP ->
sigmoid gate -> per-point scale), SPMD across 8 NeuronCores.

Sharding: batch_ids is sorted with B=16 segments; core k owns batches 2k and
2k+1 (whole batches per device). Each batch range is padded to a fixed R points
by replicating the first row of the batch (max-safe); the extra rows' sum
contribution is subtracted via a host-computed correction term.

v3: x ships as fp16 (tolerance 2e-2 dwarfs fp16 rounding; HBM bytes halve),
segment sums accumulate on the PE into f32 PSUM, and each range is processed
fully (pool -> MLP -> scale -> multiply -> store) while resident in SBUF, with
the next range reusing the same SBUF slots. x is read from HBM exactly once:
64 MB/core total traffic. Output returns fp16 and is upcast on host.
"""

import sys

for _p in ("/opt/trn_rl_repo", "/root/.axon_site/_ro/trn_rl_repo"):
    if _p not in sys.path:
        sys.path.append(_p)

import numpy as np

import concourse.bacc as bacc
import concourse.tile as tile
from concourse import bass, mybir
from concourse.bass_utils import run_bass_kernel_spmd
from concourse.masks import make_identity

NCORES = 8
B = 16
C = 64
H = 32
RPC = 2  # batch ranges per core
TP = 4096  # points per tile
F = (TP // 128) * C  # free elems per partition per tile (2048)
DT = mybir.dt.float32
DT16 = mybir.dt.float16

CHUNK_TILES = 4  # tiles per DMA chunk (16384 points = 2 MB fp16)
XBUFS = 3
# Bytes/partition of SBUF available for pinned chunks. Usable is ~208 KiB;
# reserve xpool (3x16K) + m_acc (16K) + consts/small + margin.
PIN_BUDGET = 132 * 1024


def build_nc(R: int):
    nc = bacc.Bacc("TRN2", target_bir_lowering=False, debug=False,
                   num_devices=NCORES, enable_asserts=False)

    xs = nc.dram_tensor("xs", [RPC, R, C], DT16, kind="ExternalInput")
    corrt = nc.dram_tensor("corrt", [C, RPC], DT, kind="ExternalInput")
    invct = nc.dram_tensor("invct", [C, RPC], DT, kind="ExternalInput")
    w1t = nc.dram_tensor("w1t", [C, H], DT, kind="ExternalInput")
    b1c = nc.dram_tensor("b1c", [H, 1], DT, kind="ExternalInput")
    w2t = nc.dram_tensor("w2t", [H, C], DT, kind="ExternalInput")
    b2x2 = nc.dram_tensor("b2x2", [C, 1], DT, kind="ExternalInput")
    out = nc.dram_tensor("out", [RPC, R, C], DT16, kind="ExternalOutput")

    def dram_chunk_ap(handle, r, off, npts):
        return handle.ap()[r, off:off + npts, :].rearrange(
            "(p a) c -> p (a c)", p=128)

    chunks = []
    off = 0
    while off < R:
        npts = min(CHUNK_TILES * TP, R - off)
        chunks.append((off, npts))
        off += npts
    FCMAX = CHUNK_TILES * F  # fp16 elems per partition per chunk (8192)

    # chunks [0, npin) live in dedicated SBUF slots from load to store (reused
    # across ranges); chunks >= npin stream through xpool and are re-read in
    # phase 2.
    budget = PIN_BUDGET
    npin = 0
    for ci, (off, npts) in enumerate(chunks):
        need = npts * C * 2 // 128
        if budget < need:
            break
        budget -= need
        npin = ci + 1

    with tile.TileContext(nc) as tc:
        with (
            tc.tile_pool(name="const", bufs=1) as const,
            tc.tile_pool(name="xpool", bufs=XBUFS) as xpool,
            tc.tile_pool(name="pinp", bufs=1) as pinp,
            tc.tile_pool(name="accs", bufs=1) as accs,
            tc.tile_pool(name="small", bufs=1) as small,
            tc.tile_pool(name="psum_t", bufs=1, space="PSUM") as psum_t,
            tc.tile_pool(name="psum_w", bufs=1, space="PSUM") as psum_w,
        ):
            # constants
            ident = const.tile([128, 128], DT)
            make_identity(nc, ident[:])
            ones_row = const.tile([1, 128], DT)
            nc.vector.memset(ones_row[:], 1.0)
            ones_h = const.tile([128, 1], DT16)
            nc.vector.memset(ones_h[:], 1.0)
            corrt_sb = const.tile([C, RPC], DT)
            nc.sync.dma_start(out=corrt_sb[:], in_=corrt.ap())
            invct_sb = const.tile([C, RPC], DT)
            nc.sync.dma_start(out=invct_sb[:], in_=invct.ap())
            w1t_sb = const.tile([C, H], DT)
            nc.sync.dma_start(out=w1t_sb[:], in_=w1t.ap())
            b1c_sb = const.tile([H, 1], DT)
            nc.sync.dma_start(out=b1c_sb[:], in_=b1c.ap())
            w2t_sb = const.tile([H, C], DT)
            nc.sync.dma_start(out=w2t_sb[:], in_=w2t.ap())
            b2x2_sb = const.tile([C, 1], DT)
            nc.sync.dma_start(out=b2x2_sb[:], in_=b2x2.ap())

            for r in range(RPC):
                # ---- phase 1: running max + PE sum over streamed chunks ----
                m_acc = accs.tile([128, FCMAX], DT16, tag="m_acc")
                nc.vector.memset(m_acc[:], -60000.0)
                ps_s = psum_t.tile([1, 512], DT, tag=f"ps_s{r}")
                nmm = sum(-(-npts * C // 128) // 512 for _, npts in chunks)
                mmi = 0
                resident = []
                for ci, (off, npts) in enumerate(chunks):
                    fc = npts * C // 128
                    if ci < npin:
                        xt = pinp.tile([128, fc], DT16, tag=f"pin{ci}",
                                       name=f"pin{ci}")
                        resident.append(xt)
                    else:
                        xt = xpool.tile([128, FCMAX], DT16, tag="xt")
                        resident.append(None)
                    nc.sync.dma_start(out=xt[:, :fc],
                                      in_=dram_chunk_ap(xs, r, off, npts))
                    nc.vector.tensor_max(m_acc[:, :fc], m_acc[:, :fc],
                                         xt[:, :fc])
                    for j in range(fc // 512):
                        nc.tensor.matmul(
                            out=ps_s[:], lhsT=ones_h[:],
                            rhs=xt[:, j * 512:(j + 1) * 512],
                            start=(mmi == 0), stop=(mmi == nmm - 1))
                        mmi += 1

                # ---- fold sums: [1,512] -> [1,C] -> [C,1] ----
                rhs2 = small.tile([C, 2], DT, tag=f"rhs2_{r}")
                sum_col = small.tile([C, 1], DT, tag=f"sum_col{r}")
                sum_row = small.tile([1, C], DT, tag=f"sum_row{r}")
                nc.vector.reduce_sum(
                    out=sum_row[:],
                    in_=ps_s[:].rearrange("p (a c) -> p c a", c=C),
                    axis=mybir.AxisListType.X)
                sc_ps = psum_w.tile([C, 1], DT, tag="sc")
                nc.tensor.transpose(out=sc_ps[:], in_=sum_row[:],
                                    identity=ident[:1, :1])
                nc.vector.tensor_copy(sum_col[:], sc_ps[:])

                # ---- fold max: free fold then partition fold ----
                m64 = small.tile([128, C], DT, tag="m64")
                nc.vector.reduce_max(
                    out=m64[:],
                    in_=m_acc[:].rearrange("p (a c) -> p c a", c=C),
                    axis=mybir.AxisListType.X)
                mrow_t = psum_t.tile([C, 128], DT, tag="tr")
                nc.tensor.transpose(out=mrow_t[:], in_=m64[:], identity=ident[:])
                nc.vector.reduce_max(out=rhs2[:, 1:2], in_=mrow_t[:],
                                     axis=mybir.AxisListType.X)

                # avg = (sum - corr) * invc
                nc.vector.tensor_sub(sum_col[:], sum_col[:], corrt_sb[:, r:r + 1])
                nc.vector.tensor_mul(rhs2[:, 0:1], sum_col[:],
                                     invct_sb[:, r:r + 1])

                # ---- tiny MLP: scale = 1 + sigmoid(mlp(avg) + mlp(mx)) ----
                h_ps = psum_w.tile([H, 2], DT, tag="mm")
                nc.tensor.matmul(out=h_ps[:], lhsT=w1t_sb[:], rhs=rhs2[:],
                                 start=True, stop=True)
                h_sb = small.tile([H, 2], DT, tag=f"h_sb{r}")
                nc.scalar.activation(out=h_sb[:], in_=h_ps[:],
                                     func=mybir.ActivationFunctionType.Relu,
                                     bias=b1c_sb[:])
                z_ps = psum_w.tile([C, 2], DT, tag="mm")
                nc.tensor.matmul(out=z_ps[:], lhsT=w2t_sb[:], rhs=h_sb[:],
                                 start=True, stop=True)
                z_sb = small.tile([C, 2], DT, tag=f"z_sb{r}")
                nc.vector.tensor_copy(z_sb[:], z_ps[:])
                zsum = small.tile([C, 1], DT, tag=f"zsum{r}")
                nc.vector.tensor_add(zsum[:], z_sb[:, 0:1], z_sb[:, 1:2])
                scale_c = small.tile([C, 1], DT, tag=f"scale{r}")
                nc.scalar.activation(out=scale_c[:], in_=zsum[:],
                                     func=mybir.ActivationFunctionType.Sigmoid,
                                     bias=b2x2_sb[:])
                nc.vector.tensor_scalar_add(scale_c[:], scale_c[:], 1.0)

                # broadcast scale column to [128, C] fp16
                row_ps = psum_w.tile([1, C], DT, tag="row")
                nc.tensor.transpose(out=row_ps[:], in_=scale_c[:],
                                    identity=ident[:C, :C])
                row_sb = small.tile([1, C], DT, tag=f"row_sb{r}")
                nc.vector.tensor_copy(row_sb[:], row_ps[:])
                bcast_ps = psum_w.tile([128, C], DT, tag="bc")
                nc.tensor.matmul(out=bcast_ps[:], lhsT=ones_row[:], rhs=row_sb[:],
                                 start=True, stop=True)
                mult = small.tile([128, C], DT16, tag=f"mult{r}", name=f"mult{r}")
                nc.vector.tensor_copy(mult[:], bcast_ps[:])

                # ---- phase 2: out = x * scale, from SBUF where resident ----
                for ci, (off, npts) in enumerate(chunks):
                    fa = npts // 128
                    fc = fa * C
                    mult_bc = mult[:].unsqueeze(1).to_broadcast([128, fa, C])
                    if resident[ci] is not None:
                        xt = resident[ci][:, :fc].rearrange(
                            "p (a c) -> p a c", c=C)
                    else:
                        xt_t = xpool.tile([128, CHUNK_TILES * TP // 128, C],
                                          DT16, tag="xt")
                        nc.sync.dma_start(out=xt_t[:, :fa, :],
                                          in_=dram_chunk_ap(xs, r, off, npts))
                        xt = xt_t[:, :fa, :]
                    nc.vector.tensor_mul(xt, xt, mult_bc)
                    nc.scalar.dma_start(out=dram_chunk_ap(out, r, off, npts),
                                        in_=xt)

    nc.compile()
    return nc


_CACHE: dict[int, object] = {}


def kernel(x, batch_ids, W1, b1, W2, b2):
    x = np.asarray(x, dtype=np.float32)
    batch_ids = np.asarray(batch_ids, dtype=np.int32)
    W1 = np.asarray(W1, dtype=np.float32)
    b1 = np.asarray(b1, dtype=np.float32)
    W2 = np.asarray(W2, dtype=np.float32)
    b2 = np.asarray(b2, dtype=np.float32)

    N = x.shape[0]
    x16 = x.astype(np.float16)
    bounds = np.searchsorted(batch_ids, np.arange(B + 1), side="left")
    counts = np.diff(bounds)
    R = max(TP, int(-(-counts.max() // TP)) * TP)

    nc = _CACHE.get(R)
    if nc is None:
        nc = _CACHE[R] = build_nc(R)

    xp = np.empty((NCORES, RPC, R, C), np.float16)
    corrt = np.zeros((NCORES, C, RPC), np.float32)
    invct = np.zeros((NCORES, C, RPC), np.float32)
    for b in range(B):
        core, r = divmod(b, RPC)
        s, e = int(bounds[b]), int(bounds[b + 1])
        n = e - s
        xp[core, r, :n] = x16[s:e]
        pad = x16[s] if n > 0 else np.zeros(C, np.float16)
        xp[core, r, n:] = pad
        corrt[core, :, r] = (np.float64(R - n) * pad.astype(np.float64)).astype(
            np.float32)
        invct[core, :, r] = 1.0 / max(n, 1)

    w1t = np.ascontiguousarray(W1.T)  # [C, H]
    b1c = np.ascontiguousarray(b1.reshape(H, 1))
    w2t = np.ascontiguousarray(W2.T)  # [H, C]
    b2x2 = np.ascontiguousarray((2.0 * b2).reshape(C, 1))

    in_maps = [
        {
            "xs": xp[core],
            "corrt": np.ascontiguousarray(corrt[core]),
            "invct": np.ascontiguousarray(invct[core]),
            "w1t": w1t,
            "b1c": b1c,
            "w2t": w2t,
            "b2x2": b2x2,
        }
        for core in range(NCORES)
    ]

    res = run_bass_kernel_spmd(nc, in_maps, core_ids=list(range(NCORES)))

    out = np.empty((N, C), np.float32)
    for b in range(B):
        core, r = divmod(b, RPC)
        s, e = int(bounds[b]), int(bounds[b + 1])
        out[s:e] = res.results[core]["out"][r, : e - s].astype(np.float32)
    return out


# revision 6
# speedup vs baseline: 1.0567x; 1.0567x over previous
"""Trainium2 Bass kernel for ChannelAttentionModel (segment avg/max -> tiny MLP ->
sigmoid gate -> per-point scale), SPMD across 8 NeuronCores.

Sharding: batch_ids is sorted with B=16 segments; core k owns batches 2k and
2k+1 (whole batches per device). Each batch range is padded to a fixed R points
by replicating the first row of the batch (max-safe); the extra rows' sum
contribution is subtracted via a host-computed correction term.

v4: x ships as fp16 (tolerance 2e-2 dwarfs fp16 rounding; HBM bytes halve),
segment sums accumulate on the PE into f32 PSUM, and each range is processed
fully (pool -> MLP -> scale -> multiply -> store) while resident in SBUF; the
next range reuses the same SBUF slots as they free up, plus a few dedicated
slots so its loads start before the previous range's stores finish. x is read
from HBM exactly once (64 MB/core). Scale critical path kept short: narrow max
accumulator with dense folds, memsets and ACT tables preloaded at t=0, const
DMAs ride the (initially idle) store queue.
"""

import sys

for _p in ("/opt/trn_rl_repo", "/root/.axon_site/_ro/trn_rl_repo"):
    if _p not in sys.path:
        sys.path.append(_p)

import numpy as np

import concourse.bacc as bacc
import concourse.tile as tile
from concourse import bass, mybir
from concourse.bass_utils import run_bass_kernel_spmd
from concourse.masks import make_identity

NCORES = 8
B = 16
C = 64
H = 32
RPC = 2  # batch ranges per core
TP = 4096  # points per tile
F = (TP // 128) * C  # free elems per partition per tile (2048)
DT = mybir.dt.float32
DT16 = mybir.dt.float16

CHUNK_TILES = 4  # tiles per DMA chunk (16384 points = 2 MB fp16)
XBUFS = 3
MW = 2048  # max-accumulator width (free elems per partition, multiple of C)
NEXTRA = 3  # dedicated early slots for the second range
# Bytes/partition of SBUF available for pinned chunks (incl. extra slots).
PIN_BUDGET = 176 * 1024


def build_nc(R: int):
    nc = bacc.Bacc("TRN2", target_bir_lowering=False, debug=False,
                   num_devices=NCORES, enable_asserts=False)

    xs = nc.dram_tensor("xs", [RPC, R, C], DT16, kind="ExternalInput")
    corrt = nc.dram_tensor("corrt", [C, RPC], DT, kind="ExternalInput")
    invct = nc.dram_tensor("invct", [C, RPC], DT, kind="ExternalInput")
    w1t = nc.dram_tensor("w1t", [C, H], DT, kind="ExternalInput")
    b1c = nc.dram_tensor("b1c", [H, 1], DT, kind="ExternalInput")
    w2t = nc.dram_tensor("w2t", [H, C], DT, kind="ExternalInput")
    b2x2 = nc.dram_tensor("b2x2", [C, 1], DT, kind="ExternalInput")
    out = nc.dram_tensor("out", [RPC, R, C], DT16, kind="ExternalOutput")

    def dram_chunk_ap(handle, r, off, npts):
        return handle.ap()[r, off:off + npts, :].rearrange(
            "(p a) c -> p (a c)", p=128)

    chunks = []
    off = 0
    while off < R:
        npts = min(CHUNK_TILES * TP, R - off)
        chunks.append((off, npts))
        off += npts
    FCMAX = CHUNK_TILES * F  # fp16 elems per partition per chunk (8192)
    nchunks = len(chunks)

    # Chunks [0, npin) of range 0 live in dedicated SBUF slots from load to
    # store. Range 1 reuses those slots as range 0's stores free them, with
    # NEXTRA dedicated slots so its first loads need not wait at all.
    budget = PIN_BUDGET - NEXTRA * (FCMAX * 2)
    npin = 0
    for ci, (off, npts) in enumerate(chunks):
        need = npts * C * 2 // 128
        if budget < need:
            break
        budget -= need
        npin = ci + 1
    fully = npin == nchunks
    nextra = NEXTRA if fully else 0

    def slot_tag(r, ci):
        if r == 0 or not fully:
            return f"pin{ci}"
        if ci < nextra:
            return f"pinx{ci}"
        return f"pin{ci - nextra}"

    with tile.TileContext(nc) as tc:
        with (
            tc.tile_pool(name="const", bufs=1) as const,
            tc.tile_pool(name="xpool", bufs=XBUFS) as xpool,
            tc.tile_pool(name="pinp", bufs=1) as pinp,
            tc.tile_pool(name="accs", bufs=1) as accs,
            tc.tile_pool(name="small", bufs=1) as small,
            tc.tile_pool(name="psum_t", bufs=1, space="PSUM") as psum_t,
            tc.tile_pool(name="psum_w", bufs=1, space="PSUM") as psum_w,
        ):
            # constants (DMAs on the scalar/store queue, which is idle early)
            ident = const.tile([128, 128], DT)
            make_identity(nc, ident[:])
            ones_row = const.tile([1, 128], DT)
            nc.vector.memset(ones_row[:], 1.0)
            ones_h = const.tile([128, 1], DT16)
            nc.vector.memset(ones_h[:], 1.0)
            corrt_sb = const.tile([C, RPC], DT)
            nc.scalar.dma_start(out=corrt_sb[:], in_=corrt.ap())
            invct_sb = const.tile([C, RPC], DT)
            nc.scalar.dma_start(out=invct_sb[:], in_=invct.ap())
            w1t_sb = const.tile([C, H], DT)
            nc.scalar.dma_start(out=w1t_sb[:], in_=w1t.ap())
            b1c_sb = const.tile([H, 1], DT)
            nc.scalar.dma_start(out=b1c_sb[:], in_=b1c.ap())
            w2t_sb = const.tile([H, C], DT)
            nc.scalar.dma_start(out=w2t_sb[:], in_=w2t.ap())
            b2x2_sb = const.tile([C, 1], DT)
            nc.scalar.dma_start(out=b2x2_sb[:], in_=b2x2.ap())

            # preload ACT tables (Relu, Sigmoid) off the critical path
            zz = const.tile([1, 1], DT)
            nc.vector.memset(zz[:], 0.0)
            dum = const.tile([1, 1], DT)
            nc.scalar.activation(out=dum[:], in_=zz[:],
                                 func=mybir.ActivationFunctionType.Relu,
                                 bias=zz[:])
            nc.scalar.activation(out=dum[:], in_=zz[:],
                                 func=mybir.ActivationFunctionType.Sigmoid,
                                 bias=zz[:])

            # max accumulators for both ranges, cleared up front
            m_accs = []
            for r in range(RPC):
                m_acc = accs.tile([128, MW], DT16, tag=f"m_acc{r}",
                                  name=f"m_acc{r}")
                nc.vector.memset(m_acc[:], -60000.0)
                m_accs.append(m_acc)

            for r in range(RPC):
                # ---- phase 1: running max + PE sum over streamed chunks ----
                m_acc = m_accs[r]
                ps_s = psum_t.tile([1, 512], DT, tag=f"ps_s{r}")
                nmm = sum(-(-npts * C // 128) // 512 for _, npts in chunks)
                mmi = 0
                resident = []
                for ci, (off, npts) in enumerate(chunks):
                    fc = npts * C // 128
                    if ci < npin:
                        tag = slot_tag(r, ci)
                        xt = pinp.tile([128, fc], DT16, tag=tag,
                                       name=f"{tag}_r{r}")
                        resident.append(xt)
                    else:
                        xt = xpool.tile([128, FCMAX], DT16, tag="xt")
                        resident.append(None)
                    nc.sync.dma_start(out=xt[:, :fc],
                                      in_=dram_chunk_ap(xs, r, off, npts))
                    for j in range(fc // MW):
                        nc.vector.tensor_max(m_acc[:], m_acc[:],
                                             xt[:, j * MW:(j + 1) * MW])
                    for j in range(fc // 512):
                        nc.tensor.matmul(
                            out=ps_s[:], lhsT=ones_h[:],
                            rhs=xt[:, j * 512:(j + 1) * 512],
                            start=(mmi == 0), stop=(mmi == nmm - 1))
                        mmi += 1

                # ---- fold sums: [1,512] -> [1,C] -> [C,1] ----
                rhs2 = small.tile([C, 2], DT, tag=f"rhs2_{r}")
                sum_col = small.tile([C, 1], DT, tag=f"sum_col{r}")
                sum_row = small.tile([1, C], DT, tag=f"sum_row{r}")
                nc.vector.reduce_sum(
                    out=sum_row[:],
                    in_=ps_s[:].rearrange("p (a c) -> p c a", c=C),
                    axis=mybir.AxisListType.X)
                sc_ps = psum_w.tile([C, 1], DT, tag="sc")
                nc.tensor.transpose(out=sc_ps[:], in_=sum_row[:],
                                    identity=ident[:1, :1])
                nc.vector.tensor_copy(sum_col[:], sc_ps[:])

                # ---- fold max: dense halving folds, then partition fold ----
                w = MW // 2
                while w >= C:
                    nc.vector.tensor_max(m_acc[:, :w], m_acc[:, :w],
                                         m_acc[:, w:2 * w])
                    w //= 2
                mfin = small.tile([128, C], DT, tag="mfin")
                nc.vector.tensor_copy(mfin[:], m_acc[:, :C])
                mrow_t = psum_t.tile([C, 128], DT, tag="tr")
                nc.tensor.transpose(out=mrow_t[:], in_=mfin[:], identity=ident[:])
                nc.vector.reduce_max(out=rhs2[:, 1:2], in_=mrow_t[:],
                                     axis=mybir.AxisListType.X)

                # avg = (sum - corr) * invc
                nc.vector.tensor_sub(sum_col[:], sum_col[:], corrt_sb[:, r:r + 1])
                nc.vector.tensor_mul(rhs2[:, 0:1], sum_col[:],
                                     invct_sb[:, r:r + 1])

                # ---- tiny MLP: scale = 1 + sigmoid(mlp(avg) + mlp(mx)) ----
                h_ps = psum_w.tile([H, 2], DT, tag="mm")
                nc.tensor.matmul(out=h_ps[:], lhsT=w1t_sb[:], rhs=rhs2[:],
                                 start=True, stop=True)
                h_sb = small.tile([H, 2], DT, tag=f"h_sb{r}")
                nc.scalar.activation(out=h_sb[:], in_=h_ps[:],
                                     func=mybir.ActivationFunctionType.Relu,
                                     bias=b1c_sb[:])
                z_ps = psum_w.tile([C, 2], DT, tag="mm")
                nc.tensor.matmul(out=z_ps[:], lhsT=w2t_sb[:], rhs=h_sb[:],
                                 start=True, stop=True)
                z_sb = small.tile([C, 2], DT, tag=f"z_sb{r}")
                nc.vector.tensor_copy(z_sb[:], z_ps[:])
                zsum = small.tile([C, 1], DT, tag=f"zsum{r}")
                nc.vector.tensor_add(zsum[:], z_sb[:, 0:1], z_sb[:, 1:2])
                scale_c = small.tile([C, 1], DT, tag=f"scale{r}")
                nc.scalar.activation(out=scale_c[:], in_=zsum[:],
                                     func=mybir.ActivationFunctionType.Sigmoid,
                                     bias=b2x2_sb[:])
                nc.vector.tensor_scalar_add(scale_c[:], scale_c[:], 1.0)

                # broadcast scale column to [128, C] fp16
                row_ps = psum_w.tile([1, C], DT, tag="row")
                nc.tensor.transpose(out=row_ps[:], in_=scale_c[:],
                                    identity=ident[:C, :C])
                row_sb = small.tile([1, C], DT, tag=f"row_sb{r}")
                nc.vector.tensor_copy(row_sb[:], row_ps[:])
                bcast_ps = psum_w.tile([128, C], DT, tag="bc")
                nc.tensor.matmul(out=bcast_ps[:], lhsT=ones_row[:], rhs=row_sb[:],
                                 start=True, stop=True)
                mult = small.tile([128, C], DT16, tag=f"mult{r}", name=f"mult{r}")
                nc.vector.tensor_copy(mult[:], bcast_ps[:])

                # ---- phase 2: out = x * scale, from SBUF where resident ----
                for ci, (off, npts) in enumerate(chunks):
                    fa = npts // 128
                    fc = fa * C
                    mult_bc = mult[:].unsqueeze(1).to_broadcast([128, fa, C])
                    if resident[ci] is not None:
                        xt = resident[ci][:, :fc].rearrange(
                            "p (a c) -> p a c", c=C)
                    else:
                        xt_t = xpool.tile([128, CHUNK_TILES * TP // 128, C],
                                          DT16, tag="xt")
                        nc.sync.dma_start(out=xt_t[:, :fa, :],
                                          in_=dram_chunk_ap(xs, r, off, npts))
                        xt = xt_t[:, :fa, :]
                    nc.vector.tensor_mul(xt, xt, mult_bc)
                    nc.scalar.dma_start(out=dram_chunk_ap(out, r, off, npts),
                                        in_=xt)

    nc.compile()
    return nc


_CACHE: dict[int, object] = {}


def kernel(x, batch_ids, W1, b1, W2, b2):
    x = np.asarray(x, dtype=np.float32)
    batch_ids = np.asarray(batch_ids, dtype=np.int32)
    W1 = np.asarray(W1, dtype=np.float32)
    b1 = np.asarray(b1, dtype=np.float32)
    W2 = np.asarray(W2, dtype=np.float32)
    b2 = np.asarray(b2, dtype=np.float32)

    N = x.shape[0]
    x16 = x.astype(np.float16)
    bounds = np.searchsorted(batch_ids, np.arange(B + 1), side="left")
    counts = np.diff(bounds)
    R = max(TP, int(-(-counts.max() // TP)) * TP)

    nc = _CACHE.get(R)
    if nc is None:
        nc = _CACHE[R] = build_nc(R)

    xp = np.empty((NCORES, RPC, R, C), np.float16)
    corrt = np.zeros((NCORES, C, RPC), np.float32)
    invct = np.zeros((NCORES, C, RPC), np.float32)
    for b in range(B):
        core, r = divmod(b, RPC)
        s, e = int(bounds[b]), int(bounds[b + 1])
        n = e - s
        xp[core, r, :n] = x16[s:e]
        pad = x16[s] if n > 0 else np.zeros(C, np.float16)
        xp[core, r, n:] = pad
        corrt[core, :, r] = (np.float64(R - n) * pad.astype(np.float64)).astype(
            np.float32)
        invct[core, :, r] = 1.0 / max(n, 1)

    w1t = np.ascontiguousarray(W1.T)  # [C, H]
    b1c = np.ascontiguousarray(b1.reshape(H, 1))
    w2t = np.ascontiguousarray(W2.T)  # [H, C]
    b2x2 = np.ascontiguousarray((2.0 * b2).reshape(C, 1))

    in_maps = [
        {
            "xs": xp[core],
            "corrt": np.ascontiguousarray(corrt[core]),
            "invct": np.ascontiguousarray(invct[core]),
            "w1t": w1t,
            "b1c": b1c,
            "w2t": w2t,
            "b2x2": b2x2,
        }
        for core in range(NCORES)
    ]

    res = run_bass_kernel_spmd(nc, in_maps, core_ids=list(range(NCORES)))

    out = np.empty((N, C), np.float32)
    for b in range(B):
        core, r = divmod(b, RPC)
        s, e = int(bounds[b]), int(bounds[b + 1])
        out[s:e] = res.results[core]["out"][r, : e - s].astype(np.float32)
    return out


# revision 7
# speedup vs baseline: 1.1018x; 1.0427x over previous
"""Trainium2 Bass kernel for ChannelAttentionModel (segment avg/max -> tiny MLP ->
sigmoid gate -> per-point scale), SPMD across 8 NeuronCores.

Sharding: batch_ids is sorted with B=16 segments; core k owns batches 2k and
2k+1 (whole batches per device). Each batch range is padded to a fixed R points
by replicating the first row of the batch (max-safe); the extra rows' sum
contribution is subtracted via a host-computed correction term.

v5: x ships as fp16 (tolerance 2e-2 dwarfs fp16 rounding; HBM bytes halve) and
each range is processed fully (pool -> MLP -> scale -> multiply -> store)
while resident in SBUF; the next range reuses the slots as stores free them,
plus dedicated slots so its first loads start immediately. x is read from HBM
exactly once (64 MB/core). Segment sums accumulate per-chunk on the PE into
f32 PSUM and fold into SBUF right away, so no engine ever waits on a
whole-range accumulation group; the scale tail is a short row-oriented chain
with the +1 fused into the broadcast matmul.
"""

import sys

for _p in ("/opt/trn_rl_repo", "/root/.axon_site/_ro/trn_rl_repo"):
    if _p not in sys.path:
        sys.path.append(_p)

import numpy as np

import concourse.bacc as bacc
import concourse.tile as tile
from concourse import bass, mybir
from concourse.bass_utils import run_bass_kernel_spmd
from concourse.masks import make_identity

NCORES = 8
B = 16
C = 64
H = 32
RPC = 2  # batch ranges per core
TP = 4096  # points per tile
F = (TP // 128) * C  # free elems per partition per tile (2048)
DT = mybir.dt.float32
DT16 = mybir.dt.float16

CHUNK_TILES = 4  # tiles per DMA chunk (16384 points = 2 MB fp16)
XBUFS = 3
MW = 2048  # max-accumulator width (free elems per partition, multiple of C)
NEXTRA = 4  # dedicated early slots for the second range
# Bytes/partition of SBUF available for pinned chunks (incl. extra slots).
PIN_BUDGET = 192 * 1024


def build_nc(R: int):
    nc = bacc.Bacc("TRN2", target_bir_lowering=False, debug=False,
                   num_devices=NCORES, enable_asserts=False)

    xs = nc.dram_tensor("xs", [RPC, R, C], DT16, kind="ExternalInput")
    corrt = nc.dram_tensor("corrt", [C, RPC], DT, kind="ExternalInput")
    invct = nc.dram_tensor("invct", [C, RPC], DT, kind="ExternalInput")
    w1t = nc.dram_tensor("w1t", [C, H], DT, kind="ExternalInput")
    b1c = nc.dram_tensor("b1c", [H, 1], DT, kind="ExternalInput")
    w2t = nc.dram_tensor("w2t", [H, C], DT, kind="ExternalInput")
    b2row = nc.dram_tensor("b2row", [1, C], DT, kind="ExternalInput")
    out = nc.dram_tensor("out", [RPC, R, C], DT16, kind="ExternalOutput")

    def dram_chunk_ap(handle, r, off, npts):
        return handle.ap()[r, off:off + npts, :].rearrange(
            "(p a) c -> p (a c)", p=128)

    chunks = []
    off = 0
    while off < R:
        npts = min(CHUNK_TILES * TP, R - off)
        chunks.append((off, npts))
        off += npts
    FCMAX = CHUNK_TILES * F  # fp16 elems per partition per chunk (8192)
    nchunks = len(chunks)

    # Chunks [0, npin) of range 0 live in dedicated SBUF slots from load to
    # store. Range 1 reuses those slots as range 0's stores free them, with
    # NEXTRA dedicated slots so its first loads need not wait at all.
    budget = PIN_BUDGET - NEXTRA * (FCMAX * 2)
    npin = 0
    for ci, (off, npts) in enumerate(chunks):
        need = npts * C * 2 // 128
        if budget < need:
            break
        budget -= need
        npin = ci + 1
    fully = npin == nchunks
    nextra = NEXTRA if fully else 0

    def slot_tag(r, ci):
        if r == 0 or not fully:
            return f"pin{ci}"
        if ci < nextra:
            return f"pinx{ci}"
        return f"pin{ci - nextra}"

    with tile.TileContext(nc) as tc:
        with (
            tc.tile_pool(name="const", bufs=1) as const,
            tc.tile_pool(name="xpool", bufs=XBUFS) as xpool,
            tc.tile_pool(name="pinp", bufs=1) as pinp,
            tc.tile_pool(name="accs", bufs=1) as accs,
            tc.tile_pool(name="small", bufs=1) as small,
            tc.tile_pool(name="psum_c", bufs=2, space="PSUM") as psum_c,
            tc.tile_pool(name="psum_w", bufs=1, space="PSUM") as psum_w,
        ):
            # constants (DMAs on the scalar/store queue, which is idle early)
            ident = const.tile([128, 128], DT)
            make_identity(nc, ident[:])
            ones_row = const.tile([1, 128], DT)
            nc.vector.memset(ones_row[:], 1.0)
            ones_h = const.tile([128, 1], DT16)
            nc.vector.memset(ones_h[:], 1.0)
            corrt_sb = const.tile([C, RPC], DT)
            nc.scalar.dma_start(out=corrt_sb[:], in_=corrt.ap())
            invct_sb = const.tile([C, RPC], DT)
            nc.scalar.dma_start(out=invct_sb[:], in_=invct.ap())
            w1t_sb = const.tile([C, H], DT)
            nc.scalar.dma_start(out=w1t_sb[:], in_=w1t.ap())
            b1c_sb = const.tile([H, 1], DT)
            nc.scalar.dma_start(out=b1c_sb[:], in_=b1c.ap())
            w2t_sb = const.tile([H, C], DT)
            nc.scalar.dma_start(out=w2t_sb[:], in_=w2t.ap())
            b2row_sb = const.tile([1, C], DT)
            nc.scalar.dma_start(out=b2row_sb[:], in_=b2row.ap())

            # preload ACT tables (Relu, Sigmoid) off the critical path
            zz = const.tile([1, 1], DT)
            nc.vector.memset(zz[:], 0.0)
            dum = const.tile([1, 1], DT)
            nc.scalar.activation(out=dum[:], in_=zz[:],
                                 func=mybir.ActivationFunctionType.Relu,
                                 bias=zz[:])
            nc.scalar.activation(out=dum[:], in_=zz[:],
                                 func=mybir.ActivationFunctionType.Sigmoid,
                                 bias=zz[:])

            # max accumulators for both ranges, cleared up front
            m_accs = []
            for r in range(RPC):
                m_acc = accs.tile([128, MW], DT16, tag=f"m_acc{r}",
                                  name=f"m_acc{r}")
                nc.vector.memset(m_acc[:], -60000.0)
                m_accs.append(m_acc)

            for r in range(RPC):
                # ---- phase 1: running max + per-chunk PE sums -> SBUF ----
                m_acc = m_accs[r]
                s_acc = small.tile([1, 512], DT, tag=f"s_acc{r}",
                                   name=f"s_acc{r}")
                resident = []
                for ci, (off, npts) in enumerate(chunks):
                    fc = npts * C // 128
                    if ci < npin:
                        tag = slot_tag(r, ci)
                        xt = pinp.tile([128, fc], DT16, tag=tag,
                                       name=f"{tag}_r{r}")
                        resident.append(xt)
                    else:
                        xt = xpool.tile([128, FCMAX], DT16, tag="xt")
                        resident.append(None)
                    nc.sync.dma_start(out=xt[:, :fc],
                                      in_=dram_chunk_ap(xs, r, off, npts))
                    for j in range(fc // MW):
                        nc.vector.tensor_max(m_acc[:], m_acc[:],
                                             xt[:, j * MW:(j + 1) * MW])
                    ps_c = psum_c.tile([1, 512], DT, tag="ps_c")
                    nmm = fc // 512
                    for j in range(nmm):
                        nc.tensor.matmul(
                            out=ps_c[:], lhsT=ones_h[:],
                            rhs=xt[:, j * 512:(j + 1) * 512],
                            start=(j == 0), stop=(j == nmm - 1))
                    if ci == 0:
                        nc.vector.tensor_copy(s_acc[:], ps_c[:])
                    else:
                        nc.vector.tensor_add(s_acc[:], s_acc[:], ps_c[:])

                # ---- fold sums: [1,512] -> [1,C] -> [C,1] ----
                rhs2 = small.tile([C, 2], DT, tag=f"rhs2_{r}")
                sum_col = small.tile([C, 1], DT, tag=f"sum_col{r}")
                sum_row = small.tile([1, C], DT, tag=f"sum_row{r}")
                nc.vector.reduce_sum(
                    out=sum_row[:],
                    in_=s_acc[:].rearrange("p (a c) -> p c a", c=C),
                    axis=mybir.AxisListType.X)
                sc_ps = psum_w.tile([C, 1], DT, tag="sc")
                nc.tensor.transpose(out=sc_ps[:], in_=sum_row[:],
                                    identity=ident[:1, :1])
                nc.vector.tensor_copy(sum_col[:], sc_ps[:])

                # ---- fold max: dense halving folds, then partition fold ----
                w = MW // 2
                while w >= C:
                    nc.vector.tensor_max(m_acc[:, :w], m_acc[:, :w],
                                         m_acc[:, w:2 * w])
                    w //= 2
                mfin = small.tile([128, C], DT, tag="mfin")
                nc.vector.tensor_copy(mfin[:], m_acc[:, :C])
                mrow_t = psum_w.tile([C, 128], DT, tag="tr")
                nc.tensor.transpose(out=mrow_t[:], in_=mfin[:], identity=ident[:])
                nc.vector.reduce_max(out=rhs2[:, 1:2], in_=mrow_t[:],
                                     axis=mybir.AxisListType.X)

                # avg = (sum - corr) * invc
                nc.vector.tensor_sub(sum_col[:], sum_col[:], corrt_sb[:, r:r + 1])
                nc.vector.tensor_mul(rhs2[:, 0:1], sum_col[:],
                                     invct_sb[:, r:r + 1])

                # ---- tiny MLP: scale = 1 + sigmoid(mlp(avg) + mlp(mx)) ----
                h_ps = psum_w.tile([H, 2], DT, tag="mm")
                nc.tensor.matmul(out=h_ps[:], lhsT=w1t_sb[:], rhs=rhs2[:],
                                 start=True, stop=True)
                h_sb = small.tile([H, 2], DT, tag=f"h_sb{r}")
                nc.scalar.activation(out=h_sb[:], in_=h_ps[:],
                                     func=mybir.ActivationFunctionType.Relu,
                                     bias=b1c_sb[:])
                hsum = small.tile([H, 1], DT, tag=f"hsum{r}")
                nc.vector.tensor_add(hsum[:], h_sb[:, 0:1], h_sb[:, 1:2])
                z_ps = psum_w.tile([1, C], DT, tag="zrow")
                nc.tensor.matmul(out=z_ps[:], lhsT=hsum[:], rhs=w2t_sb[:],
                                 start=True, stop=True)
                z_sb = small.tile([1, C], DT, tag=f"z_sb{r}")
                nc.vector.tensor_add(z_sb[:], z_ps[:], b2row_sb[:])
                scale_row = small.tile([1, C], DT, tag=f"scale{r}")
                nc.scalar.activation(out=scale_row[:], in_=z_sb[:],
                                     func=mybir.ActivationFunctionType.Sigmoid,
                                     bias=zz[:])
                # mult = broadcast(scale_row) + 1, via two accumulating matmuls
                bcast_ps = psum_w.tile([128, C], DT, tag="bc")
                nc.tensor.matmul(out=bcast_ps[:], lhsT=ones_row[:],
                                 rhs=scale_row[:], start=True, stop=False)
                nc.tensor.matmul(out=bcast_ps[:], lhsT=ones_row[:],
                                 rhs=ones_row[:1, :C], start=False, stop=True)
                mult = small.tile([128, C], DT16, tag=f"mult{r}", name=f"mult{r}")
                nc.vector.tensor_copy(mult[:], bcast_ps[:])

                # ---- phase 2: out = x * scale, from SBUF where resident ----
                for ci, (off, npts) in enumerate(chunks):
                    fa = npts // 128
                    fc = fa * C
                    mult_bc = mult[:].unsqueeze(1).to_broadcast([128, fa, C])
                    if resident[ci] is not None:
                        xt = resident[ci][:, :fc].rearrange(
                            "p (a c) -> p a c", c=C)
                    else:
                        xt_t = xpool.tile([128, CHUNK_TILES * TP // 128, C],
                                          DT16, tag="xt")
                        nc.sync.dma_start(out=xt_t[:, :fa, :],
                                          in_=dram_chunk_ap(xs, r, off, npts))
                        xt = xt_t[:, :fa, :]
                    nc.vector.tensor_mul(xt, xt, mult_bc)
                    nc.scalar.dma_start(out=dram_chunk_ap(out, r, off, npts),
                                        in_=xt)

    nc.compile()
    return nc


_CACHE: dict[int, object] = {}


def kernel(x, batch_ids, W1, b1, W2, b2):
    x = np.asarray(x, dtype=np.float32)
    batch_ids = np.asarray(batch_ids, dtype=np.int32)
    W1 = np.asarray(W1, dtype=np.float32)
    b1 = np.asarray(b1, dtype=np.float32)
    W2 = np.asarray(W2, dtype=np.float32)
    b2 = np.asarray(b2, dtype=np.float32)

    N = x.shape[0]
    x16 = x.astype(np.float16)
    bounds = np.searchsorted(batch_ids, np.arange(B + 1), side="left")
    counts = np.diff(bounds)
    R = max(TP, int(-(-counts.max() // TP)) * TP)

    nc = _CACHE.get(R)
    if nc is None:
        nc = _CACHE[R] = build_nc(R)

    xp = np.empty((NCORES, RPC, R, C), np.float16)
    corrt = np.zeros((NCORES, C, RPC), np.float32)
    invct = np.zeros((NCORES, C, RPC), np.float32)
    for b in range(B):
        core, r = divmod(b, RPC)
        s, e = int(bounds[b]), int(bounds[b + 1])
        n = e - s
        xp[core, r, :n] = x16[s:e]
        pad = x16[s] if n > 0 else np.zeros(C, np.float16)
        xp[core, r, n:] = pad
        corrt[core, :, r] = (np.float64(R - n) * pad.astype(np.float64)).astype(
            np.float32)
        invct[core, :, r] = 1.0 / max(n, 1)

    w1t = np.ascontiguousarray(W1.T)  # [C, H]
    b1c = np.ascontiguousarray(b1.reshape(H, 1))
    w2t = np.ascontiguousarray(W2.T)  # [H, C]
    b2row = np.ascontiguousarray((2.0 * b2).reshape(1, C))

    in_maps = [
        {
            "xs": xp[core],
            "corrt": np.ascontiguousarray(corrt[core]),
            "invct": np.ascontiguousarray(invct[core]),
            "w1t": w1t,
            "b1c": b1c,
            "w2t": w2t,
            "b2row": b2row,
        }
        for core in range(NCORES)
    ]

    res = run_bass_kernel_spmd(nc, in_maps, core_ids=list(range(NCORES)))

    out = np.empty((N, C), np.float32)
    for b in range(B):
        core, r = divmod(b, RPC)
        s, e = int(bounds[b]), int(bounds[b + 1])
        out[s:e] = res.results[core]["out"][r, : e - s].astype(np.float32)
    return out


# revision 11
# speedup vs baseline: 1.2271x; 1.1137x over previous
"""Trainium2 Bass kernel for ChannelAttentionModel (segment avg/max -> tiny MLP ->
sigmoid gate -> per-point scale), SPMD across 8 NeuronCores.

Sharding: batch_ids is sorted with B=16 segments; core k owns batches 2k and
2k+1 (whole batches per device). Each batch range is padded to a fixed R points
by replicating the first row of the batch (max-safe); the extra rows' sum
contribution is subtracted via a host-computed correction term.

v5: x ships as fp16 (tolerance 2e-2 dwarfs fp16 rounding; HBM bytes halve) and
each range is processed fully (pool -> MLP -> scale -> multiply -> store)
while resident in SBUF; the next range reuses the slots as stores free them,
plus dedicated slots so its first loads start immediately. x is read from HBM
exactly once (64 MB/core). Segment sums accumulate per-chunk on the PE into
f32 PSUM and fold into SBUF right away, so no engine ever waits on a
whole-range accumulation group; the scale tail is a short row-oriented chain
with the +1 fused into the broadcast matmul.
"""

import sys

for _p in ("/opt/trn_rl_repo", "/root/.axon_site/_ro/trn_rl_repo"):
    if _p not in sys.path:
        sys.path.append(_p)

import numpy as np

import concourse.bacc as bacc
import concourse.tile as tile
from concourse import bass, mybir
from concourse.bass_utils import run_bass_kernel_spmd
from concourse.masks import make_identity

NCORES = 8
B = 16
C = 64
H = 32
RPC = 2  # batch ranges per core
TP = 4096  # points per tile
F = (TP // 128) * C  # free elems per partition per tile (2048)
DT = mybir.dt.float32
DT16 = mybir.dt.float16

CHUNK_TILES = 4  # tiles per DMA chunk (16384 points = 2 MB fp16)
XBUFS = 3
MW = 2048  # max-accumulator width (free elems per partition, multiple of C)
NEXTRA = 4  # dedicated early slots for the second range
# Bytes/partition of SBUF available for pinned chunks (incl. extra slots).
PIN_BUDGET = 192 * 1024


def build_nc(R: int):
    nc = bacc.Bacc("TRN2", target_bir_lowering=False, debug=False,
                   num_devices=NCORES, enable_asserts=False)

    xs = nc.dram_tensor("xs", [RPC, R, C], DT16, kind="ExternalInput")
    corrt = nc.dram_tensor("corrt", [C, RPC], DT, kind="ExternalInput")
    invct = nc.dram_tensor("invct", [C, RPC], DT, kind="ExternalInput")
    w1t = nc.dram_tensor("w1t", [C, H], DT, kind="ExternalInput")
    b1c = nc.dram_tensor("b1c", [H, 1], DT, kind="ExternalInput")
    w2t = nc.dram_tensor("w2t", [H, C], DT, kind="ExternalInput")
    b2row = nc.dram_tensor("b2row", [1, C], DT, kind="ExternalInput")
    out = nc.dram_tensor("out", [RPC, R, C], DT16, kind="ExternalOutput")

    def dram_chunk_ap(handle, r, off, npts):
        return handle.ap()[r, off:off + npts, :].rearrange(
            "(p a) c -> p (a c)", p=128)

    chunks = []
    off = 0
    while off < R:
        npts = min(CHUNK_TILES * TP, R - off)
        chunks.append((off, npts))
        off += npts
    FCMAX = CHUNK_TILES * F  # fp16 elems per partition per chunk (8192)
    nchunks = len(chunks)

    # Chunks [0, npin) of range 0 live in dedicated SBUF slots from load to
    # store. Range 1 reuses those slots as range 0's stores free them, with
    # NEXTRA dedicated slots so its first loads need not wait at all.
    budget = PIN_BUDGET - NEXTRA * (FCMAX * 2)
    npin = 0
    for ci, (off, npts) in enumerate(chunks):
        need = npts * C * 2 // 128
        if budget < need:
            break
        budget -= need
        npin = ci + 1
    fully = npin == nchunks
    nextra = NEXTRA if fully else 0

    def slot_tag(r, ci):
        if r == 0 or not fully:
            return f"pin{ci}"
        if ci < nextra:
            return f"pinx{ci}"
        return f"pin{ci - nextra}"

    with tile.TileContext(nc) as tc:
        with (
            tc.tile_pool(name="const", bufs=1) as const,
            tc.tile_pool(name="xpool", bufs=XBUFS) as xpool,
            tc.tile_pool(name="pinp", bufs=1) as pinp,
            tc.tile_pool(name="accs", bufs=1) as accs,
            tc.tile_pool(name="small", bufs=1) as small,
            tc.tile_pool(name="psum_c", bufs=2, space="PSUM") as psum_c,
            tc.tile_pool(name="psum_w", bufs=1, space="PSUM") as psum_w,
        ):
            # constants (DMAs on the scalar/store queue, which is idle early)
            ident = const.tile([128, 128], DT)
            make_identity(nc, ident[:])
            ones_row = const.tile([1, 128], DT)
            nc.vector.memset(ones_row[:], 1.0)
            ones_h = const.tile([128, 1], DT16)
            nc.vector.memset(ones_h[:], 1.0)
            corrt_sb = const.tile([C, RPC], DT)
            nc.scalar.dma_start(out=corrt_sb[:], in_=corrt.ap())
            invct_sb = const.tile([C, RPC], DT)
            nc.scalar.dma_start(out=invct_sb[:], in_=invct.ap())
            w1t_sb = const.tile([C, H], DT)
            nc.scalar.dma_start(out=w1t_sb[:], in_=w1t.ap())
            b1c_sb = const.tile([H, 1], DT)
            nc.scalar.dma_start(out=b1c_sb[:], in_=b1c.ap())
            w2t_sb = const.tile([H, C], DT)
            nc.scalar.dma_start(out=w2t_sb[:], in_=w2t.ap())
            b2row_sb = const.tile([1, C], DT)
            nc.scalar.dma_start(out=b2row_sb[:], in_=b2row.ap())

            # preload the ACT sigmoid table off the critical path
            zz = const.tile([1, 1], DT)
            nc.vector.memset(zz[:], 0.0)
            one11 = const.tile([1, 1], DT)
            nc.vector.memset(one11[:], 1.0)
            dum = const.tile([1, 1], DT)
            nc.scalar.activation(out=dum[:], in_=zz[:],
                                 func=mybir.ActivationFunctionType.Sigmoid,
                                 bias=zz[:])

            # max accumulators for both ranges, cleared up front
            m_accs = []
            for r in range(RPC):
                m_acc = accs.tile([128, MW], DT16, tag=f"m_acc{r}",
                                  name=f"m_acc{r}")
                nc.vector.memset(m_acc[:], -60000.0)
                m_accs.append(m_acc)

            for r in range(RPC):
                # ---- phase 1: running max + per-chunk PE sums -> SBUF ----
                m_acc = m_accs[r]
                s_acc = small.tile([1, 512], DT, tag=f"s_acc{r}",
                                   name=f"s_acc{r}")
                resident = []
                for ci, (off, npts) in enumerate(chunks):
                    fc = npts * C // 128
                    if ci < npin:
                        tag = slot_tag(r, ci)
                        xt = pinp.tile([128, fc], DT16, tag=tag,
                                       name=f"{tag}_r{r}")
                        resident.append(xt)
                    else:
                        xt = xpool.tile([128, FCMAX], DT16, tag="xt")
                        resident.append(None)
                    nc.sync.dma_start(out=xt[:, :fc],
                                      in_=dram_chunk_ap(xs, r, off, npts))
                    for j in range(fc // MW):
                        nc.vector.tensor_max(m_acc[:], m_acc[:],
                                             xt[:, j * MW:(j + 1) * MW])
                    ps_c = psum_c.tile([1, 512], DT, tag="ps_c")
                    nmm = fc // 512
                    for j in range(nmm):
                        nc.tensor.matmul(
                            out=ps_c[:], lhsT=ones_h[:],
                            rhs=xt[:, j * 512:(j + 1) * 512],
                            start=(j == 0), stop=(j == nmm - 1))
                    if ci == 0:
                        nc.vector.tensor_copy(s_acc[:], ps_c[:])
                    else:
                        nc.vector.tensor_add(s_acc[:], s_acc[:], ps_c[:])

                # ---- fold stats into one [128,128] tile, single transpose ----
                # cols 0..C-1: per-partition channel max; row 0 cols C..2C-1:
                # per-channel sums.
                rhs2 = small.tile([C, 2], DT, tag=f"rhs2_{r}")
                T = small.tile([128, 2 * C], DT, tag=f"T{r}", name=f"T{r}")
                nc.vector.reduce_sum(
                    out=T[0:1, C:2 * C],
                    in_=s_acc[:].rearrange("p (a c) -> p c a", c=C),
                    axis=mybir.AxisListType.X)
                w = MW // 2
                while w >= C:
                    nc.vector.tensor_max(m_acc[:, :w], m_acc[:, :w],
                                         m_acc[:, w:2 * w])
                    w //= 2
                nc.vector.tensor_copy(T[:, :C], m_acc[:, :C])
                TR = psum_w.tile([2 * C, 128], DT, tag="tr")
                nc.tensor.transpose(out=TR[:], in_=T[:], identity=ident[:])
                nc.vector.reduce_max(out=rhs2[:, 1:2], in_=TR[0:C, :],
                                     axis=mybir.AxisListType.X)
                # avg = (sum - corr) * invc
                sum_col = small.tile([C, 1], DT, tag=f"sum_col{r}")
                nc.vector.tensor_sub(sum_col[:], TR[C:2 * C, 0:1],
                                     corrt_sb[:, r:r + 1])
                nc.vector.tensor_mul(rhs2[:, 0:1], sum_col[:],
                                     invct_sb[:, r:r + 1])

                # ---- tiny MLP: scale = 1 + sigmoid(mlp(avg) + mlp(mx)) ----
                h_ps = psum_w.tile([H, 2], DT, tag="mm")
                nc.tensor.matmul(out=h_ps[:], lhsT=w1t_sb[:], rhs=rhs2[:],
                                 start=True, stop=True)
                h_sb = small.tile([H, 2], DT, tag=f"h_sb{r}")
                nc.vector.tensor_scalar(out=h_sb[:], in0=h_ps[:],
                                        scalar1=b1c_sb[:], scalar2=0.0,
                                        op0=mybir.AluOpType.add,
                                        op1=mybir.AluOpType.max)
                hsum = small.tile([H, 1], DT, tag=f"hsum{r}")
                nc.vector.tensor_add(hsum[:], h_sb[:, 0:1], h_sb[:, 1:2])
                z_ps = psum_w.tile([1, C], DT, tag="zrow")
                nc.tensor.matmul(out=z_ps[:], lhsT=hsum[:], rhs=w2t_sb[:],
                                 start=True, stop=False)
                nc.tensor.matmul(out=z_ps[:], lhsT=one11[:], rhs=b2row_sb[:],
                                 start=False, stop=True)
                scale_row = small.tile([1, C], DT, tag=f"scale{r}")
                nc.scalar.activation(out=scale_row[:], in_=z_ps[:],
                                     func=mybir.ActivationFunctionType.Sigmoid,
                                     bias=zz[:])
                # mult = broadcast(scale_row) + 1, via two accumulating matmuls
                bcast_ps = psum_w.tile([128, C], DT, tag="bc")
                nc.tensor.matmul(out=bcast_ps[:], lhsT=ones_row[:],
                                 rhs=scale_row[:], start=True, stop=False)
                nc.tensor.matmul(out=bcast_ps[:], lhsT=ones_row[:],
                                 rhs=ones_row[:1, :C], start=False, stop=True)
                mult = small.tile([128, C], DT16, tag=f"mult{r}", name=f"mult{r}")
                nc.vector.tensor_copy(mult[:], bcast_ps[:])

                # ---- phase 2: out = x * scale, from SBUF where resident ----
                for ci, (off, npts) in enumerate(chunks):
                    fa = npts // 128
                    fc = fa * C
                    if resident[ci] is not None:
                        xt = resident[ci][:, :fc].rearrange(
                            "p (a c) -> p a c", c=C)
                    else:
                        xt_t = xpool.tile([128, CHUNK_TILES * TP // 128, C],
                                          DT16, tag="xt")
                        nc.sync.dma_start(out=xt_t[:, :fa, :],
                                          in_=dram_chunk_ap(xs, r, off, npts))
                        xt = xt_t[:, :fa, :]
                    # the first chunk is split so its first store issues as
                    # early as possible after the scale is ready
                    nsplit = 4 if (ci == 0 and fa % 4 == 0) else 1
                    fs = fa // nsplit
                    for k in range(nsplit):
                        sl = xt[:, k * fs:(k + 1) * fs, :]
                        mult_bc = mult[:].unsqueeze(1).to_broadcast(
                            [128, fs, C])
                        nc.vector.tensor_mul(sl, sl, mult_bc)
                        out_ap = out.ap()[r, off:off + npts, :].rearrange(
                            "(p a) c -> p a c", p=128)[:, k * fs:(k + 1) * fs, :]
                        nc.scalar.dma_start(out=out_ap, in_=sl)

    nc.compile()
    return nc


_CACHE: dict[int, object] = {}


def kernel(x, batch_ids, W1, b1, W2, b2):
    x = np.asarray(x, dtype=np.float32)
    batch_ids = np.asarray(batch_ids, dtype=np.int32)
    W1 = np.asarray(W1, dtype=np.float32)
    b1 = np.asarray(b1, dtype=np.float32)
    W2 = np.asarray(W2, dtype=np.float32)
    b2 = np.asarray(b2, dtype=np.float32)

    N = x.shape[0]
    x16 = x.astype(np.float16)
    bounds = np.searchsorted(batch_ids, np.arange(B + 1), side="left")
    counts = np.diff(bounds)
    R = max(TP, int(-(-counts.max() // TP)) * TP)

    nc = _CACHE.get(R)
    if nc is None:
        nc = _CACHE[R] = build_nc(R)

    xp = np.empty((NCORES, RPC, R, C), np.float16)
    corrt = np.zeros((NCORES, C, RPC), np.float32)
    invct = np.zeros((NCORES, C, RPC), np.float32)
    for b in range(B):
        core, r = divmod(b, RPC)
        s, e = int(bounds[b]), int(bounds[b + 1])
        n = e - s
        xp[core, r, :n] = x16[s:e]
        pad = x16[s] if n > 0 else np.zeros(C, np.float16)
        xp[core, r, n:] = pad
        corrt[core, :, r] = (np.float64(R - n) * pad.astype(np.float64)).astype(
            np.float32)
        invct[core, :, r] = 1.0 / max(n, 1)

    w1t = np.ascontiguousarray(W1.T)  # [C, H]
    b1c = np.ascontiguousarray(b1.reshape(H, 1))
    w2t = np.ascontiguousarray(W2.T)  # [H, C]
    b2row = np.ascontiguousarray((2.0 * b2).reshape(1, C))

    in_maps = [
        {
            "xs": xp[core],
            "corrt": np.ascontiguousarray(corrt[core]),
            "invct": np.ascontiguousarray(invct[core]),
            "w1t": w1t,
            "b1c": b1c,
            "w2t": w2t,
            "b2row": b2row,
        }
        for core in range(NCORES)
    ]

    res = run_bass_kernel_spmd(nc, in_maps, core_ids=list(range(NCORES)))

    out = np.empty((N, C), np.float32)
    for b in range(B):
        core, r = divmod(b, RPC)
        s, e = int(bounds[b]), int(bounds[b + 1])
        out[s:e] = res.results[core]["out"][r, : e - s].astype(np.float32)
    return out
